# revision 4
# baseline (speedup 1.0000x reference)
"""Tensor-parallel full-attention Bass kernel for TRN2 (v2, mostly-bf16).

Sharding: 16 heads over 8 cores (2 heads/core). Each core computes its heads'
QKV projections, rope, full attention, and its partial output projection
(rows of Wo for its heads); the host sums the 8 partial outputs.

v2 changes vs v1 (549us):
  - bf16 for x, all weights, v, e=exp(s), oT and y partials (half DMA/SBUF,
    FWL weight loads). qT/kT stay f32r for score precision.
  - softmax denominator: DVE/GPSIMD adder tree over e tiles + ONE ones-matmul
    per (h, i-block) instead of a ones-matmul per j-tile (-51us PE).
  - exp over [128,1024] psum pairs (fewer ACT instructions).
  - output-projection (C) matmul groups are software-pipelined one i-block
    late and interleaved between score/PV groups so PE never waits on ACT.
  - y stores + rope swaps on gpsimd DMA queue; x/weights on sync queue so
    the next batch's x prefetches during attention.
  - phase A per block ordered K, V, Q so kT is complete before the last
    Q-rope, letting B start with no PE gap.

Per-core layouts (tokens on the free axis):
  xT   [D=2048, B*T=4096] bf16  x transposed (host-prepped), replicated
  wq/wk [2048, 256] bf16        head-column shard; within each head the 128
                                columns are permuted evens-then-odds so rope
                                pairs become contiguous partition halves
  wv   [2048, 256] bf16         natural column shard
  wo   [256, 2048] bf16         natural row shard
  cs1  [128, 2048] f32          [cos.T ; sin.T] stacked
  cs2  [128, 2048] f32          [sin.T ; cos.T]
"""

import sys

sys.path.insert(0, "/opt/trn_rl_repo")

import numpy as np
import ml_dtypes

import concourse.bass as bass
import concourse.mybir as mybir
import concourse.tile as tile
from concourse import bacc
from concourse.bass_utils import run_bass_kernel_spmd

B, T, D = 2, 2048, 2048
NH, HD = 16, 128
NCORES = 8
HPC = NH // NCORES          # heads per core = 2
CPC = HPC * HD              # proj columns per core = 256
BT = B * T                  # 4096 tokens
P = 128
TBLK = 512                  # phase-A token block
NBLK = T // TBLK            # 4 blocks per batch
DC = D // P                 # 16 contraction chunks
IBLK = 512                  # phase-B query block
NIB = T // IBLK             # 4 i-blocks per batch
NJT = T // P                # 16 key tiles per batch
NJP = NJT // 2              # 8 key-tile pairs
SCALE = 1.0 / float(np.sqrt(HD))

f32 = mybir.dt.float32
f32r = mybir.dt.float32r
bf16 = mybir.dt.bfloat16

_compiled = {}

# exposed for test.py
last_results = None


def _build():
    nc = bacc.Bacc("TRN2", target_bir_lowering=False, debug=False)

    xT_d = nc.dram_tensor("xT", [D, BT], bf16, kind="ExternalInput").ap()
    wq_d = nc.dram_tensor("wq", [D, CPC], bf16, kind="ExternalInput").ap()
    wk_d = nc.dram_tensor("wk", [D, CPC], bf16, kind="ExternalInput").ap()
    wv_d = nc.dram_tensor("wv", [D, CPC], bf16, kind="ExternalInput").ap()
    wo_d = nc.dram_tensor("wo", [CPC, D], bf16, kind="ExternalInput").ap()
    cs1_d = nc.dram_tensor("cs1", [P, T], f32, kind="ExternalInput").ap()
    cs2_d = nc.dram_tensor("cs2", [P, T], f32, kind="ExternalInput").ap()
    y_d = nc.dram_tensor("y", [BT, D], bf16, kind="ExternalOutput").ap()

    with tile.TileContext(nc) as tc:
        _emit(nc, tc, xT_d, wq_d, wk_d, wv_d, wo_d, cs1_d, cs2_d, y_d)
    nc.compile()
    return nc


def _emit(nc, tc, xT_d, wq_d, wk_d, wv_d, wo_d, cs1_d, cs2_d, y_d):
    from contextlib import ExitStack

    Exp = mybir.ActivationFunctionType.Exp
    mult = mybir.AluOpType.mult
    add = mybir.AluOpType.add
    sub = mybir.AluOpType.subtract

    with ExitStack() as ctx:
        const = ctx.enter_context(tc.tile_pool(name="const", bufs=1))
        state = ctx.enter_context(tc.tile_pool(name="state", bufs=1))

        wq_sb = const.tile([P, DC * CPC], bf16, tag="wq")
        wk_sb = const.tile([P, DC * CPC], bf16, tag="wk")
        wv_sb = const.tile([P, DC * CPC], bf16, tag="wv")
        wo_sb = const.tile([P, HPC * D], bf16, tag="wo")
        cs1_sb = const.tile([P, T], f32, tag="cs1")
        cs2_sb = const.tile([P, T], f32, tag="cs2")
        ones_sb = const.tile([P, P], bf16, tag="ones")

        nc.sync.dma_start(
            wq_sb[:].rearrange("p (dc c) -> p dc c", dc=DC),
            wq_d.rearrange("(dc p) c -> p dc c", p=P))
        nc.sync.dma_start(
            wk_sb[:].rearrange("p (dc c) -> p dc c", dc=DC),
            wk_d.rearrange("(dc p) c -> p dc c", p=P))
        nc.sync.dma_start(
            wv_sb[:].rearrange("p (dc c) -> p dc c", dc=DC),
            wv_d.rearrange("(dc p) c -> p dc c", p=P))
        nc.sync.dma_start(
            wo_sb[:].rearrange("p (h d) -> p h d", h=HPC),
            wo_d.rearrange("(h p) d -> p h d", p=P))
        nc.sync.dma_start(cs1_sb[:], cs1_d[:])
        nc.sync.dma_start(cs2_sb[:], cs2_d[:])
        nc.gpsimd.memset(ones_sb[:], 1.0)

        qT_sb = state.tile([P, HPC * T], f32r, tag="qT")
        kT_sb = state.tile([P, HPC * T], f32r, tag="kT")
        v_sb = state.tile([P, NJT * CPC], bf16, tag="v")

        xpool = ctx.enter_context(tc.tile_pool(name="xa", bufs=2))
        yps = ctx.enter_context(tc.tile_pool(name="y_ps", bufs=2, space="PSUM"))
        ypool = ctx.enter_context(tc.tile_pool(name="yb", bufs=3))
        opool = ctx.enter_context(tc.tile_pool(name="ob", bufs=4))

        def make_cgroup(g0, tt, oT0, oT1, tl, db):
            def emit():
                yp = yps.tile([P, IBLK], f32, tag="y")
                nc.tensor.matmul(
                    yp[:], oT0[:, tl * P:(tl + 1) * P],
                    wo_sb[:, db * IBLK:(db + 1) * IBLK],
                    start=True, stop=False)
                nc.tensor.matmul(
                    yp[:], oT1[:, tl * P:(tl + 1) * P],
                    wo_sb[:, D + db * IBLK:D + (db + 1) * IBLK],
                    start=False, stop=True)
                yt = ypool.tile([P, IBLK], bf16, tag="yt")
                nc.vector.tensor_copy(out=yt[:], in_=yp[:])
                nc.gpsimd.dma_start(
                    y_d[g0 + tt * P:g0 + (tt + 1) * P,
                        db * IBLK:(db + 1) * IBLK],
                    yt[:])
            return emit

        pending_c = []

        def rope(rpool, pps, t0, dst, h):
            m1 = rpool.tile([P, TBLK], f32, tag="m1")
            m3 = rpool.tile([P, TBLK], f32, tag="m3")
            c1 = cs1_sb[:, t0:t0 + TBLK]
            c2 = cs2_sb[:, t0:t0 + TBLK]
            nc.vector.tensor_tensor(m1[:], pps[:], c1, mult)
            nc.vector.tensor_tensor(m3[:], pps[:], c2, mult)
            sw = rpool.tile([P, TBLK], f32, tag="sw")
            nc.gpsimd.dma_start(sw[0:64, :], m1[64:128, :])
            nc.gpsimd.dma_start(sw[64:128, :], m3[0:64, :])
            o = dst[:, h * T + t0:h * T + t0 + TBLK]
            nc.vector.tensor_tensor(o[0:64, :], m1[0:64, :], sw[0:64, :], sub)
            nc.vector.tensor_tensor(o[64:128, :], m3[64:128, :], sw[64:128, :], add)

        for b in range(B):
            g0 = b * T

            with tc.tile_pool(name=f"ra{b}", bufs=4) as rpool, \
                 tc.tile_pool(name=f"qk_ps{b}", bufs=4, space="PSUM") as qkps, \
                 tc.tile_pool(name=f"v_ps{b}", bufs=2, space="PSUM") as vps:
                for blk in range(NBLK):
                    t0 = blk * TBLK
                    xt = xpool.tile([P, DC * TBLK], bf16, tag="x")
                    nc.sync.dma_start(
                        xt[:].rearrange("p (dc t) -> p dc t", dc=DC),
                        xT_d[:, g0 + t0:g0 + t0 + TBLK]
                        .rearrange("(dc p) t -> p dc t", p=P))

                    if blk == 0 and pending_c:
                        # leftover C groups of the previous batch fill the
                        # x-prefetch window at this batch's start
                        for cg in pending_c:
                            cg()
                        pending_c = []

                    # K first so kT is complete before the last Q rope
                    for w_sb, dst in ((wk_sb, kT_sb), (None, None), (wq_sb, qT_sb)):
                        if w_sb is None:
                            # V-projection: natural [token, col] tiles
                            for half in range(2):
                                vp = vps.tile([P, 2 * CPC], f32, tag="v")
                                for tl2 in range(2):
                                    tl = half * 2 + tl2
                                    for dc in range(DC):
                                        nc.tensor.matmul(
                                            vp[:, tl2 * CPC:(tl2 + 1) * CPC],
                                            xt[:, dc * TBLK + tl * P:
                                               dc * TBLK + (tl + 1) * P],
                                            wv_sb[:, dc * CPC:(dc + 1) * CPC],
                                            start=(dc == 0), stop=(dc == DC - 1))
                                nc.scalar.copy(
                                    v_sb[:, (4 * blk + 2 * half) * CPC:
                                         (4 * blk + 2 * half + 2) * CPC],
                                    vp[:])
                            continue
                        for h in range(HPC):
                            pps = qkps.tile([P, TBLK], f32, tag="qk")
                            for dc in range(DC):
                                nc.tensor.matmul(
                                    pps[:],
                                    w_sb[:, dc * CPC + h * HD:dc * CPC + (h + 1) * HD],
                                    xt[:, dc * TBLK:(dc + 1) * TBLK],
                                    start=(dc == 0), stop=(dc == DC - 1))
                            rope(rpool, pps, t0, dst, h)

            with tc.tile_pool(name=f"e{b}", bufs=3) as epool, \
                 tc.tile_pool(name=f"tr{b}", bufs=12) as tpool, \
                 tc.tile_pool(name=f"rc{b}", bufs=2) as rcpool, \
                 tc.tile_pool(name=f"s_ps{b}", bufs=2, space="PSUM") as sps, \
                 tc.tile_pool(name=f"o_ps{b}", bufs=2, space="PSUM") as ops:
                for ib in range(NIB):
                    i0 = ib * IBLK
                    cw = pending_c
                    pending_c = []
                    ci = 0
                    oTs = []
                    for h in range(HPC):
                        q_sl = qT_sb[:, h * T + i0:h * T + i0 + IBLK]
                        op = ops.tile([P, IBLK], f32, tag="o")
                        l1s = []
                        for jp in range(NJP):
                            j0 = h * T + 2 * jp * P
                            sp = sps.tile([P, 2 * IBLK], f32, tag="s")
                            nc.tensor.matmul(
                                sp[:, 0:IBLK], kT_sb[:, j0:j0 + P],
                                q_sl, start=True, stop=True)
                            nc.tensor.matmul(
                                sp[:, IBLK:2 * IBLK], kT_sb[:, j0 + P:j0 + 2 * P],
                                q_sl, start=True, stop=True)
                            e = epool.tile([P, 2 * IBLK], bf16, tag="e")
                            nc.scalar.activation(e[:], sp[:], Exp, scale=SCALE)
                            if ci < len(cw):
                                cw[ci]()
                                ci += 1
                            nc.tensor.matmul(
                                op[:],
                                v_sb[:, 2 * jp * CPC + h * HD:
                                     2 * jp * CPC + (h + 1) * HD],
                                e[:, 0:IBLK],
                                start=(jp == 0), stop=False)
                            nc.tensor.matmul(
                                op[:],
                                v_sb[:, (2 * jp + 1) * CPC + h * HD:
                                     (2 * jp + 1) * CPC + (h + 1) * HD],
                                e[:, IBLK:2 * IBLK],
                                start=False, stop=(jp == NJP - 1))
                            t1 = tpool.tile([P, IBLK], bf16, tag="t")
                            nc.gpsimd.tensor_tensor(
                                t1[:], e[:, 0:IBLK], e[:, IBLK:2 * IBLK], add)
                            l1s.append(t1)
                        while len(l1s) > 1:
                            nxt = []
                            for k in range(0, len(l1s), 2):
                                t2 = tpool.tile([P, IBLK], bf16, tag="t")
                                nc.gpsimd.tensor_tensor(
                                    t2[:], l1s[k][:], l1s[k + 1][:], add)
                                nxt.append(t2)
                            l1s = nxt
                        dn = sps.tile([P, 2 * IBLK], f32, tag="s")
                        nc.tensor.matmul(
                            dn[:, 0:IBLK], ones_sb[:], l1s[0][:],
                            start=True, stop=True)
                        rcp = rcpool.tile([P, IBLK], f32, tag="rc")
                        nc.vector.reciprocal_approx_fast(
                            out=rcp[:], in_=dn[:, 0:IBLK])
                        oT_h = opool.tile([P, IBLK], bf16, tag="oT")
                        nc.vector.tensor_tensor(oT_h[:], op[:], rcp[:], mult)
                        oTs.append(oT_h)
                    while ci < len(cw):
                        cw[ci]()
                        ci += 1
                    for tl in range(IBLK // P):
                        tt = ib * (IBLK // P) + tl
                        for db in range(D // IBLK):
                            pending_c.append(
                                make_cgroup(g0, tt, oTs[0], oTs[1], tl, db))

        # tail: last i-block's output projection
        for cg in pending_c:
            cg()


_EVEN_ODD = np.concatenate([np.arange(0, HD, 2), np.arange(1, HD, 2)])


def _prep_inputs(x, rope_cos, rope_sin, Wq, Wk, Wv, Wo):
    bf = ml_dtypes.bfloat16
    x = np.asarray(x, dtype=np.float32)
    xT = np.ascontiguousarray(x.reshape(BT, D).T.astype(bf))
    cosT = np.asarray(rope_cos, dtype=np.float32).T
    sinT = np.asarray(rope_sin, dtype=np.float32).T
    cs1 = np.ascontiguousarray(
        np.concatenate([cosT, sinT], axis=0), dtype=np.float32)
    cs2 = np.ascontiguousarray(
        np.concatenate([sinT, cosT], axis=0), dtype=np.float32)
    Wq = np.asarray(Wq, dtype=np.float32)
    Wk = np.asarray(Wk, dtype=np.float32)
    Wv = np.asarray(Wv, dtype=np.float32)
    Wo = np.asarray(Wo, dtype=np.float32)

    in_maps = []
    for c in range(NCORES):
        cols = slice(c * CPC, (c + 1) * CPC)
        wq_c = Wq[:, cols].reshape(D, HPC, HD)[:, :, _EVEN_ODD].reshape(D, CPC)
        wk_c = Wk[:, cols].reshape(D, HPC, HD)[:, :, _EVEN_ODD].reshape(D, CPC)
        in_maps.append({
            "xT": xT,
            "wq": np.ascontiguousarray(wq_c.astype(bf)),
            "wk": np.ascontiguousarray(wk_c.astype(bf)),
            "wv": np.ascontiguousarray(Wv[:, cols].astype(bf)),
            "wo": np.ascontiguousarray(Wo[cols, :].astype(bf)),
            "cs1": cs1,
            "cs2": cs2,
        })
    return in_maps


def kernel(x, rope_cos, rope_sin, Wq, Wk, Wv, Wo, _trace=False):
    global last_results
    if "nc" not in _compiled:
        _compiled["nc"] = _build()
    nc = _compiled["nc"]
    in_maps = _prep_inputs(x, rope_cos, rope_sin, Wq, Wk, Wv, Wo)
    res = run_bass_kernel_spmd(
        nc, in_maps, core_ids=list(range(NCORES)), trace=_trace)
    last_results = res
    y = np.sum(np.stack([res.results[c]["y"].astype(np.float32)
                         for c in range(NCORES)]),
               axis=0, dtype=np.float64)
    return y.reshape(B, T, D).astype(np.float32)


# revision 8
# speedup vs baseline: 1.2429x; 1.2429x over previous
"""Tensor-parallel full-attention Bass kernel for TRN2 (v2, mostly-bf16).

Sharding: 16 heads over 8 cores (2 heads/core). Each core computes its heads'
QKV projections, rope, full attention, and its partial output projection
(rows of Wo for its heads); the host sums the 8 partial outputs.

v2 changes vs v1 (549us):
  - bf16 for x, all weights, v, e=exp(s), oT and y partials (half DMA/SBUF,
    FWL weight loads). qT/kT stay f32r for score precision.
  - softmax denominator: DVE/GPSIMD adder tree over e tiles + ONE ones-matmul
    per (h, i-block) instead of a ones-matmul per j-tile (-51us PE).
  - exp over [128,1024] psum pairs (fewer ACT instructions).
  - output-projection (C) matmul groups are software-pipelined one i-block
    late and interleaved between score/PV groups so PE never waits on ACT.
  - y stores + rope swaps on gpsimd DMA queue; x/weights on sync queue so
    the next batch's x prefetches during attention.
  - phase A per block ordered K, V, Q so kT is complete before the last
    Q-rope, letting B start with no PE gap.

Per-core layouts (tokens on the free axis):
  xT   [D=2048, B*T=4096] bf16  x transposed (host-prepped), replicated
  wq/wk [2048, 256] bf16        head-column shard; within each head the 128
                                columns are permuted evens-then-odds so rope
                                pairs become contiguous partition halves
  wv   [2048, 256] bf16         natural column shard
  wo   [256, 2048] bf16         natural row shard
  cs1  [128, 2048] f32          [cos.T ; sin.T] stacked
  cs2  [128, 2048] f32          [sin.T ; cos.T]
"""

import sys

sys.path.insert(0, "/opt/trn_rl_repo")

import numpy as np
import ml_dtypes

import concourse.bass as bass
import concourse.mybir as mybir
import concourse.tile as tile
from concourse import bacc
from concourse.bass_utils import run_bass_kernel_spmd

B, T, D = 2, 2048, 2048
NH, HD = 16, 128
NCORES = 8
HPC = NH // NCORES          # heads per core = 2
CPC = HPC * HD              # proj columns per core = 256
BT = B * T                  # 4096 tokens
P = 128
TBLK = 512                  # phase-A token block
NBLK = T // TBLK            # 4 blocks per batch
DC = D // P                 # 16 contraction chunks
IBLK = 512                  # phase-B query block
NIB = T // IBLK             # 4 i-blocks per batch
NJT = T // P                # 16 key tiles per batch
NJP = NJT // 2              # 8 key-tile pairs
SCALE = 1.0 / float(np.sqrt(HD))

f32 = mybir.dt.float32
f32r = mybir.dt.float32r
bf16 = mybir.dt.bfloat16

_compiled = {}

# exposed for test.py
last_results = None


def _build():
    nc = bacc.Bacc("TRN2", target_bir_lowering=False, debug=False)

    xT_d = nc.dram_tensor("xT", [D, BT], bf16, kind="ExternalInput").ap()
    wq_d = nc.dram_tensor("wq", [D, CPC], bf16, kind="ExternalInput").ap()
    wk_d = nc.dram_tensor("wk", [D, CPC], bf16, kind="ExternalInput").ap()
    wv_d = nc.dram_tensor("wv", [D, CPC], bf16, kind="ExternalInput").ap()
    wo_d = nc.dram_tensor("wo", [CPC, D], bf16, kind="ExternalInput").ap()
    cs1_d = nc.dram_tensor("cs1", [P, T], f32, kind="ExternalInput").ap()
    cs2_d = nc.dram_tensor("cs2", [P, T], f32, kind="ExternalInput").ap()
    y_d = nc.dram_tensor("y", [BT, D], bf16, kind="ExternalOutput").ap()

    with tile.TileContext(nc) as tc:
        _emit(nc, tc, xT_d, wq_d, wk_d, wv_d, wo_d, cs1_d, cs2_d, y_d)
    nc.compile()
    return nc


def _emit(nc, tc, xT_d, wq_d, wk_d, wv_d, wo_d, cs1_d, cs2_d, y_d):
    from contextlib import ExitStack

    Exp = mybir.ActivationFunctionType.Exp
    mult = mybir.AluOpType.mult
    add = mybir.AluOpType.add
    sub = mybir.AluOpType.subtract

    with ExitStack() as ctx:
        const = ctx.enter_context(tc.tile_pool(name="const", bufs=1))
        state = ctx.enter_context(tc.tile_pool(name="state", bufs=1))

        wq_sb = const.tile([P, DC * CPC], bf16, tag="wq")
        wk_sb = const.tile([P, DC * CPC], bf16, tag="wk")
        wv_sb = const.tile([P, DC * CPC], bf16, tag="wv")
        wo_sb = const.tile([P, HPC * D], bf16, tag="wo")
        cs1_sb = const.tile([P, T], f32, tag="cs1")
        cs2_sb = const.tile([P, T], f32, tag="cs2")
        ones_sb = const.tile([P, P], bf16, tag="ones")

        # wk on the sync queue ahead of x block 0; the rest of the consts load
        # in parallel on the scalar engine's DMA queue.
        nc.sync.dma_start(
            wk_sb[:].rearrange("p (dc c) -> p dc c", dc=DC),
            wk_d.rearrange("(dc p) c -> p dc c", p=P))
        nc.scalar.dma_start(cs1_sb[:], cs1_d[:])
        nc.scalar.dma_start(cs2_sb[:], cs2_d[:])
        nc.scalar.dma_start(
            wv_sb[:].rearrange("p (dc c) -> p dc c", dc=DC),
            wv_d.rearrange("(dc p) c -> p dc c", p=P))
        nc.scalar.dma_start(
            wq_sb[:].rearrange("p (dc c) -> p dc c", dc=DC),
            wq_d.rearrange("(dc p) c -> p dc c", p=P))
        nc.scalar.dma_start(
            wo_sb[:].rearrange("p (h d) -> p h d", h=HPC),
            wo_d.rearrange("(h p) d -> p h d", p=P))
        nc.gpsimd.memset(ones_sb[:], 1.0)

        qT_sb = state.tile([P, HPC * T], f32r, tag="qT")
        kT_sb = state.tile([P, HPC * T], f32r, tag="kT")
        v_sb = state.tile([P, NJT * CPC], bf16, tag="v")

        xpool = ctx.enter_context(tc.tile_pool(name="xa", bufs=2))
        yps = ctx.enter_context(tc.tile_pool(name="y_ps", bufs=1, space="PSUM"))
        ypool = ctx.enter_context(tc.tile_pool(name="yb", bufs=3))
        opool = ctx.enter_context(tc.tile_pool(name="ob", bufs=4))

        def make_cgroup(g0, tt, oT0, oT1, tl, db):
            def emit():
                yp = yps.tile([P, IBLK], f32, tag="y")
                nc.tensor.matmul(
                    yp[:], oT0[:, tl * P:(tl + 1) * P],
                    wo_sb[:, db * IBLK:(db + 1) * IBLK],
                    start=True, stop=False)
                nc.tensor.matmul(
                    yp[:], oT1[:, tl * P:(tl + 1) * P],
                    wo_sb[:, D + db * IBLK:D + (db + 1) * IBLK],
                    start=False, stop=True)
                yt = ypool.tile([P, IBLK], bf16, tag="yt")
                nc.vector.tensor_copy(out=yt[:], in_=yp[:])
                nc.gpsimd.dma_start(
                    y_d[g0 + tt * P:g0 + (tt + 1) * P,
                        db * IBLK:(db + 1) * IBLK],
                    yt[:])
            return emit

        pending_c = []

        def rope(rpool, pps, t0, dst, h):
            m1 = rpool.tile([P, TBLK], f32, tag="m1")
            m3 = rpool.tile([P, TBLK], f32, tag="m3")
            c1 = cs1_sb[:, t0:t0 + TBLK]
            c2 = cs2_sb[:, t0:t0 + TBLK]
            nc.vector.tensor_tensor(m1[:], pps[:], c1, mult)
            nc.vector.tensor_tensor(m3[:], pps[:], c2, mult)
            sw = rpool.tile([P, TBLK], f32, tag="sw")
            nc.gpsimd.dma_start(sw[0:64, :], m1[64:128, :])
            nc.gpsimd.dma_start(sw[64:128, :], m3[0:64, :])
            o = dst[:, h * T + t0:h * T + t0 + TBLK]
            nc.vector.tensor_tensor(o[0:64, :], m1[0:64, :], sw[0:64, :], sub)
            nc.vector.tensor_tensor(o[64:128, :], m3[64:128, :], sw[64:128, :], add)

        for b in range(B):
            g0 = b * T

            with tc.tile_pool(name=f"ra{b}", bufs=4) as rpool, \
                 tc.tile_pool(name=f"qk_ps{b}", bufs=4, space="PSUM") as qkps, \
                 tc.tile_pool(name=f"v_ps{b}", bufs=2, space="PSUM") as vps:
                for blk in range(NBLK):
                    t0 = blk * TBLK
                    xt = xpool.tile([P, DC * TBLK], bf16, tag="x")
                    nc.sync.dma_start(
                        xt[:].rearrange("p (dc t) -> p dc t", dc=DC),
                        xT_d[:, g0 + t0:g0 + t0 + TBLK]
                        .rearrange("(dc p) t -> p dc t", p=P))

                    if blk == 0 and pending_c:
                        # leftover C groups of the previous batch fill the
                        # x-prefetch window at this batch's start
                        for cg in pending_c:
                            cg()
                        pending_c = []

                    # K first so kT is complete before the last Q rope
                    for w_sb, dst in ((wk_sb, kT_sb), (None, None), (wq_sb, qT_sb)):
                        if w_sb is None:
                            # V-projection: natural [token, col] tiles
                            for half in range(2):
                                vp = vps.tile([P, 2 * CPC], f32, tag="v")
                                for tl2 in range(2):
                                    tl = half * 2 + tl2
                                    for dc in range(DC):
                                        nc.tensor.matmul(
                                            vp[:, tl2 * CPC:(tl2 + 1) * CPC],
                                            xt[:, dc * TBLK + tl * P:
                                               dc * TBLK + (tl + 1) * P],
                                            wv_sb[:, dc * CPC:(dc + 1) * CPC],
                                            start=(dc == 0), stop=(dc == DC - 1))
                                nc.scalar.copy(
                                    v_sb[:, (4 * blk + 2 * half) * CPC:
                                         (4 * blk + 2 * half + 2) * CPC],
                                    vp[:])
                            continue
                        for h in range(HPC):
                            pps = qkps.tile([P, TBLK], f32, tag="qk")
                            for dc in range(DC):
                                nc.tensor.matmul(
                                    pps[:],
                                    w_sb[:, dc * CPC + h * HD:dc * CPC + (h + 1) * HD],
                                    xt[:, dc * TBLK:(dc + 1) * TBLK],
                                    start=(dc == 0), stop=(dc == DC - 1))
                            rope(rpool, pps, t0, dst, h)

            with tc.tile_pool(name=f"e{b}", bufs=3) as epool, \
                 tc.tile_pool(name=f"rc{b}", bufs=2) as rcpool, \
                 tc.tile_pool(name=f"s_ps{b}", bufs=2, space="PSUM") as sps, \
                 tc.tile_pool(name=f"o_ps{b}", bufs=2, space="PSUM") as ops, \
                 tc.tile_pool(name=f"d_ps{b}", bufs=1, space="PSUM") as dps:
                for ib in range(NIB):
                    i0 = ib * IBLK
                    cw = pending_c
                    pending_c = []
                    ci = 0
                    oTs = []
                    for h in range(HPC):
                        q_sl = qT_sb[:, h * T + i0:h * T + i0 + IBLK]
                        op = ops.tile([P, IBLK], f32, tag="o")
                        dn = dps.tile([P, IBLK], f32, tag="d")
                        for jp in range(NJP):
                            j0 = h * T + 2 * jp * P
                            sp = sps.tile([P, 2 * IBLK], f32, tag="s")
                            nc.tensor.matmul(
                                sp[:, 0:IBLK], kT_sb[:, j0:j0 + P],
                                q_sl, start=True, stop=True)
                            nc.tensor.matmul(
                                sp[:, IBLK:2 * IBLK], kT_sb[:, j0 + P:j0 + 2 * P],
                                q_sl, start=True, stop=True)
                            e = epool.tile([P, 2 * IBLK], bf16, tag="e")
                            nc.scalar.activation(e[:], sp[:], Exp, scale=SCALE)
                            if ci < len(cw):
                                cw[ci]()
                                ci += 1
                            nc.tensor.matmul(
                                op[:],
                                v_sb[:, 2 * jp * CPC + h * HD:
                                     2 * jp * CPC + (h + 1) * HD],
                                e[:, 0:IBLK],
                                start=(jp == 0), stop=False)
                            nc.tensor.matmul(
                                op[:],
                                v_sb[:, (2 * jp + 1) * CPC + h * HD:
                                     (2 * jp + 1) * CPC + (h + 1) * HD],
                                e[:, IBLK:2 * IBLK],
                                start=False, stop=(jp == NJP - 1))
                            nc.tensor.matmul(
                                dn[:], ones_sb[:], e[:, 0:IBLK],
                                start=(jp == 0), stop=False)
                            nc.tensor.matmul(
                                dn[:], ones_sb[:], e[:, IBLK:2 * IBLK],
                                start=False, stop=(jp == NJP - 1))
                        rcp = rcpool.tile([P, IBLK], f32, tag="rc")
                        nc.vector.reciprocal_approx_fast(out=rcp[:], in_=dn[:])
                        oT_h = opool.tile([P, IBLK], bf16, tag="oT")
                        nc.vector.tensor_tensor(oT_h[:], op[:], rcp[:], mult)
                        oTs.append(oT_h)
                    while ci < len(cw):
                        cw[ci]()
                        ci += 1
                    for tl in range(IBLK // P):
                        tt = ib * (IBLK // P) + tl
                        for db in range(D // IBLK):
                            pending_c.append(
                                make_cgroup(g0, tt, oTs[0], oTs[1], tl, db))

        # tail: last i-block's output projection
        for cg in pending_c:
            cg()


_EVEN_ODD = np.concatenate([np.arange(0, HD, 2), np.arange(1, HD, 2)])


def _prep_inputs(x, rope_cos, rope_sin, Wq, Wk, Wv, Wo):
    bf = ml_dtypes.bfloat16
    x = np.asarray(x, dtype=np.float32)
    xT = np.ascontiguousarray(x.reshape(BT, D).T.astype(bf))
    cosT = np.asarray(rope_cos, dtype=np.float32).T
    sinT = np.asarray(rope_sin, dtype=np.float32).T
    cs1 = np.ascontiguousarray(
        np.concatenate([cosT, sinT], axis=0), dtype=np.float32)
    cs2 = np.ascontiguousarray(
        np.concatenate([sinT, cosT], axis=0), dtype=np.float32)
    Wq = np.asarray(Wq, dtype=np.float32)
    Wk = np.asarray(Wk, dtype=np.float32)
    Wv = np.asarray(Wv, dtype=np.float32)
    Wo = np.asarray(Wo, dtype=np.float32)

    in_maps = []
    for c in range(NCORES):
        cols = slice(c * CPC, (c + 1) * CPC)
        wq_c = Wq[:, cols].reshape(D, HPC, HD)[:, :, _EVEN_ODD].reshape(D, CPC)
        wk_c = Wk[:, cols].reshape(D, HPC, HD)[:, :, _EVEN_ODD].reshape(D, CPC)
        in_maps.append({
            "xT": xT,
            "wq": np.ascontiguousarray(wq_c.astype(bf)),
            "wk": np.ascontiguousarray(wk_c.astype(bf)),
            "wv": np.ascontiguousarray(Wv[:, cols].astype(bf)),
            "wo": np.ascontiguousarray(Wo[cols, :].astype(bf)),
            "cs1": cs1,
            "cs2": cs2,
        })
    return in_maps


def kernel(x, rope_cos, rope_sin, Wq, Wk, Wv, Wo, _trace=False):
    global last_results
    if "nc" not in _compiled:
        _compiled["nc"] = _build()
    nc = _compiled["nc"]
    in_maps = _prep_inputs(x, rope_cos, rope_sin, Wq, Wk, Wv, Wo)
    res = run_bass_kernel_spmd(
        nc, in_maps, core_ids=list(range(NCORES)), trace=_trace)
    last_results = res
    y = np.sum(np.stack([res.results[c]["y"].astype(np.float32)
                         for c in range(NCORES)]),
               axis=0, dtype=np.float64)
    return y.reshape(B, T, D).astype(np.float32)


# revision 9
# speedup vs baseline: 1.2812x; 1.0308x over previous
"""Tensor-parallel full-attention Bass kernel for TRN2 (v2, mostly-bf16).

Sharding: 16 heads over 8 cores (2 heads/core). Each core computes its heads'
QKV projections, rope, full attention, and its partial output projection
(rows of Wo for its heads); the host sums the 8 partial outputs.

v2 changes vs v1 (549us):
  - bf16 for x, all weights, v, e=exp(s), oT and y partials (half DMA/SBUF,
    FWL weight loads). qT/kT stay f32r for score precision.
  - softmax denominator: DVE/GPSIMD adder tree over e tiles + ONE ones-matmul
    per (h, i-block) instead of a ones-matmul per j-tile (-51us PE).
  - exp over [128,1024] psum pairs (fewer ACT instructions).
  - output-projection (C) matmul groups are software-pipelined one i-block
    late and interleaved between score/PV groups so PE never waits on ACT.
  - y stores + rope swaps on gpsimd DMA queue; x/weights on sync queue so
    the next batch's x prefetches during attention.
  - phase A per block ordered K, V, Q so kT is complete before the last
    Q-rope, letting B start with no PE gap.

Per-core layouts (tokens on the free axis):
  xT   [D=2048, B*T=4096] bf16  x transposed (host-prepped), replicated
  wq/wk [2048, 256] bf16        head-column shard; within each head the 128
                                columns are permuted evens-then-odds so rope
                                pairs become contiguous partition halves
  wv   [2048, 256] bf16         natural column shard
  wo   [256, 2048] bf16         natural row shard
  cs1  [128, 2048] f32          [cos.T ; sin.T] stacked
  cs2  [128, 2048] f32          [sin.T ; cos.T]
"""

import sys

sys.path.insert(0, "/opt/trn_rl_repo")

import numpy as np
import ml_dtypes

import concourse.bass as bass
import concourse.mybir as mybir
import concourse.tile as tile
from concourse import bacc
from concourse.bass_utils import run_bass_kernel_spmd

B, T, D = 2, 2048, 2048
NH, HD = 16, 128
NCORES = 8
HPC = NH // NCORES          # heads per core = 2
CPC = HPC * HD              # proj columns per core = 256
BT = B * T                  # 4096 tokens
P = 128
TBLK = 512                  # phase-A token block
NBLK = T // TBLK            # 4 blocks per batch
DC = D // P                 # 16 contraction chunks
IBLK = 512                  # phase-B query block
NIB = T // IBLK             # 4 i-blocks per batch
NJT = T // P                # 16 key tiles per batch
NJP = NJT // 2              # 8 key-tile pairs
SCALE = 1.0 / float(np.sqrt(HD))

f32 = mybir.dt.float32
f32r = mybir.dt.float32r
bf16 = mybir.dt.bfloat16

_compiled = {}

# exposed for test.py
last_results = None


def _build():
    nc = bacc.Bacc("TRN2", target_bir_lowering=False, debug=False)

    xT_d = nc.dram_tensor("xT", [D, BT], bf16, kind="ExternalInput").ap()
    wq_d = nc.dram_tensor("wq", [D, CPC], bf16, kind="ExternalInput").ap()
    wk_d = nc.dram_tensor("wk", [D, CPC], bf16, kind="ExternalInput").ap()
    wv_d = nc.dram_tensor("wv", [D, CPC], bf16, kind="ExternalInput").ap()
    wo_d = nc.dram_tensor("wo", [CPC, D], bf16, kind="ExternalInput").ap()
    cs1_d = nc.dram_tensor("cs1", [P, T], f32, kind="ExternalInput").ap()
    cs2_d = nc.dram_tensor("cs2", [P, T], f32, kind="ExternalInput").ap()
    y_d = nc.dram_tensor("y", [BT, D], bf16, kind="ExternalOutput").ap()

    with tile.TileContext(nc) as tc:
        _emit(nc, tc, xT_d, wq_d, wk_d, wv_d, wo_d, cs1_d, cs2_d, y_d)
    nc.compile()
    return nc


def _emit(nc, tc, xT_d, wq_d, wk_d, wv_d, wo_d, cs1_d, cs2_d, y_d):
    from contextlib import ExitStack

    Exp = mybir.ActivationFunctionType.Exp
    mult = mybir.AluOpType.mult
    add = mybir.AluOpType.add
    sub = mybir.AluOpType.subtract

    with ExitStack() as ctx:
        const = ctx.enter_context(tc.tile_pool(name="const", bufs=1))
        state = ctx.enter_context(tc.tile_pool(name="state", bufs=1))

        wq_sb = const.tile([P, DC * CPC], bf16, tag="wq")
        wk_sb = const.tile([P, DC * CPC], bf16, tag="wk")
        wv_sb = const.tile([P, DC * CPC], bf16, tag="wv")
        wo_sb = const.tile([P, HPC * D], bf16, tag="wo")
        cs1_sb = const.tile([P, T], f32, tag="cs1")
        cs2_sb = const.tile([P, T], f32, tag="cs2")
        ones_sb = const.tile([P, P], bf16, tag="ones")

        # wk on the sync queue ahead of x block 0; the rest of the consts load
        # in parallel on the scalar engine's DMA queue.
        nc.sync.dma_start(
            wk_sb[:].rearrange("p (dc c) -> p dc c", dc=DC),
            wk_d.rearrange("(dc p) c -> p dc c", p=P))
        nc.scalar.dma_start(cs1_sb[:], cs1_d[:])
        nc.scalar.dma_start(cs2_sb[:], cs2_d[:])
        nc.scalar.dma_start(
            wv_sb[:].rearrange("p (dc c) -> p dc c", dc=DC),
            wv_d.rearrange("(dc p) c -> p dc c", p=P))
        nc.scalar.dma_start(
            wq_sb[:].rearrange("p (dc c) -> p dc c", dc=DC),
            wq_d.rearrange("(dc p) c -> p dc c", p=P))
        nc.scalar.dma_start(
            wo_sb[:].rearrange("p (h d) -> p h d", h=HPC),
            wo_d.rearrange("(h p) d -> p h d", p=P))
        nc.gpsimd.memset(ones_sb[:], 1.0)

        qT_sb = state.tile([P, HPC * T], bf16, tag="qT")
        kT_sb = state.tile([P, HPC * T], bf16, tag="kT")
        v_sb = state.tile([P, NJT * CPC], bf16, tag="v")

        xpool = ctx.enter_context(tc.tile_pool(name="xa", bufs=2))
        yps = ctx.enter_context(tc.tile_pool(name="y_ps", bufs=1, space="PSUM"))
        ypool = ctx.enter_context(tc.tile_pool(name="yb", bufs=3))
        opool = ctx.enter_context(tc.tile_pool(name="ob", bufs=4))
        epool = ctx.enter_context(tc.tile_pool(name="eg", bufs=3))
        rcpool = ctx.enter_context(tc.tile_pool(name="rcg", bufs=2))

        def make_cgroup(g0, tt, oT0, oT1, tl, db):
            def emit():
                yp = yps.tile([P, IBLK], f32, tag="y")
                nc.tensor.matmul(
                    yp[:], oT0[:, tl * P:(tl + 1) * P],
                    wo_sb[:, db * IBLK:(db + 1) * IBLK],
                    start=True, stop=False)
                nc.tensor.matmul(
                    yp[:], oT1[:, tl * P:(tl + 1) * P],
                    wo_sb[:, D + db * IBLK:D + (db + 1) * IBLK],
                    start=False, stop=True)
                yt = ypool.tile([P, IBLK], bf16, tag="yt")
                nc.vector.tensor_copy(out=yt[:], in_=yp[:])
                nc.gpsimd.dma_start(
                    y_d[g0 + tt * P:g0 + (tt + 1) * P,
                        db * IBLK:(db + 1) * IBLK],
                    yt[:])
            return emit

        pending_c = []

        def rope(rpool, pps, t0, dst, h):
            m1 = rpool.tile([P, TBLK], bf16, tag="m1")
            m3 = rpool.tile([P, TBLK], bf16, tag="m3")
            c1 = cs1_sb[:, t0:t0 + TBLK]
            c2 = cs2_sb[:, t0:t0 + TBLK]
            nc.vector.tensor_tensor(m1[:], pps[:], c1, mult)
            nc.vector.tensor_tensor(m3[:], pps[:], c2, mult)
            sw = rpool.tile([P, TBLK], bf16, tag="sw")
            nc.gpsimd.dma_start(sw[0:64, :], m1[64:128, :])
            nc.gpsimd.dma_start(sw[64:128, :], m3[0:64, :])
            o = dst[:, h * T + t0:h * T + t0 + TBLK]
            nc.vector.tensor_tensor(o[0:64, :], m1[0:64, :], sw[0:64, :], sub)
            nc.vector.tensor_tensor(o[64:128, :], m3[64:128, :], sw[64:128, :], add)

        for b in range(B):
            g0 = b * T

            with tc.tile_pool(name=f"ra{b}", bufs=4) as rpool, \
                 tc.tile_pool(name=f"qk_ps{b}", bufs=4, space="PSUM") as qkps, \
                 tc.tile_pool(name=f"v_ps{b}", bufs=2, space="PSUM") as vps:
                for blk in range(NBLK):
                    t0 = blk * TBLK
                    xt = xpool.tile([P, DC * TBLK], bf16, tag="x")
                    nc.sync.dma_start(
                        xt[:].rearrange("p (dc t) -> p dc t", dc=DC),
                        xT_d[:, g0 + t0:g0 + t0 + TBLK]
                        .rearrange("(dc p) t -> p dc t", p=P))

                    if blk == 0 and pending_c:
                        # leftover C groups of the previous batch fill the
                        # x-prefetch window at this batch's start
                        for cg in pending_c:
                            cg()
                        pending_c = []

                    # K first so kT is complete before the last Q rope
                    for w_sb, dst in ((wk_sb, kT_sb), (None, None), (wq_sb, qT_sb)):
                        if w_sb is None:
                            # V-projection: natural [token, col] tiles
                            for half in range(2):
                                vp = vps.tile([P, 2 * CPC], f32, tag="v")
                                for tl2 in range(2):
                                    tl = half * 2 + tl2
                                    for dc in range(DC):
                                        nc.tensor.matmul(
                                            vp[:, tl2 * CPC:(tl2 + 1) * CPC],
                                            xt[:, dc * TBLK + tl * P:
                                               dc * TBLK + (tl + 1) * P],
                                            wv_sb[:, dc * CPC:(dc + 1) * CPC],
                                            start=(dc == 0), stop=(dc == DC - 1))
                                nc.scalar.copy(
                                    v_sb[:, (4 * blk + 2 * half) * CPC:
                                         (4 * blk + 2 * half + 2) * CPC],
                                    vp[:])
                            continue
                        for h in range(HPC):
                            pps = qkps.tile([P, TBLK], f32, tag="qk")
                            for dc in range(DC):
                                nc.tensor.matmul(
                                    pps[:],
                                    w_sb[:, dc * CPC + h * HD:dc * CPC + (h + 1) * HD],
                                    xt[:, dc * TBLK:(dc + 1) * TBLK],
                                    start=(dc == 0), stop=(dc == DC - 1))
                            rope(rpool, pps, t0, dst, h)

            with tc.tile_pool(name=f"o_ps{b}", bufs=2, space="PSUM") as ops, \
                 tc.tile_pool(name=f"d_ps{b}", bufs=1, space="PSUM") as dps, \
                 tc.tile_pool(name=f"s_ps{b}", bufs=2, space="PSUM") as sps:
                for ib in range(NIB):
                    i0 = ib * IBLK
                    cw = pending_c
                    pending_c = []
                    ci = 0
                    oTs = []
                    for h in range(HPC):
                        q_sl = qT_sb[:, h * T + i0:h * T + i0 + IBLK]
                        op = ops.tile([P, IBLK], f32, tag="o")
                        dn = dps.tile([P, IBLK], f32, tag="d")
                        for jp in range(NJP):
                            j0 = h * T + 2 * jp * P
                            sp = sps.tile([P, 2 * IBLK], f32, tag="s")
                            nc.tensor.matmul(
                                sp[:, 0:IBLK], kT_sb[:, j0:j0 + P],
                                q_sl, start=True, stop=True)
                            nc.tensor.matmul(
                                sp[:, IBLK:2 * IBLK], kT_sb[:, j0 + P:j0 + 2 * P],
                                q_sl, start=True, stop=True)
                            e = epool.tile([P, 2 * IBLK], bf16, tag="e")
                            nc.scalar.activation(e[:], sp[:], Exp, scale=SCALE)
                            if ci < len(cw):
                                cw[ci]()
                                ci += 1
                            nc.tensor.matmul(
                                op[:],
                                v_sb[:, 2 * jp * CPC + h * HD:
                                     2 * jp * CPC + (h + 1) * HD],
                                e[:, 0:IBLK],
                                start=(jp == 0), stop=False)
                            nc.tensor.matmul(
                                op[:],
                                v_sb[:, (2 * jp + 1) * CPC + h * HD:
                                     (2 * jp + 1) * CPC + (h + 1) * HD],
                                e[:, IBLK:2 * IBLK],
                                start=False, stop=(jp == NJP - 1))
                            nc.tensor.matmul(
                                dn[:], ones_sb[:], e[:, 0:IBLK],
                                start=(jp == 0), stop=False)
                            nc.tensor.matmul(
                                dn[:], ones_sb[:], e[:, IBLK:2 * IBLK],
                                start=False, stop=(jp == NJP - 1))
                        rcp = rcpool.tile([P, IBLK], f32, tag="rc")
                        nc.vector.reciprocal_approx_fast(out=rcp[:], in_=dn[:])
                        oT_h = opool.tile([P, IBLK], bf16, tag="oT")
                        nc.vector.tensor_tensor(oT_h[:], op[:], rcp[:], mult)
                        oTs.append(oT_h)
                    while ci < len(cw):
                        cw[ci]()
                        ci += 1
                    for tl in range(IBLK // P):
                        tt = ib * (IBLK // P) + tl
                        for db in range(D // IBLK):
                            pending_c.append(
                                make_cgroup(g0, tt, oTs[0], oTs[1], tl, db))

        # tail: last i-block's output projection
        for cg in pending_c:
            cg()


_EVEN_ODD = np.concatenate([np.arange(0, HD, 2), np.arange(1, HD, 2)])


def _prep_inputs(x, rope_cos, rope_sin, Wq, Wk, Wv, Wo):
    bf = ml_dtypes.bfloat16
    x = np.asarray(x, dtype=np.float32)
    xT = np.ascontiguousarray(x.reshape(BT, D).T.astype(bf))
    cosT = np.asarray(rope_cos, dtype=np.float32).T
    sinT = np.asarray(rope_sin, dtype=np.float32).T
    cs1 = np.ascontiguousarray(
        np.concatenate([cosT, sinT], axis=0), dtype=np.float32)
    cs2 = np.ascontiguousarray(
        np.concatenate([sinT, cosT], axis=0), dtype=np.float32)
    Wq = np.asarray(Wq, dtype=np.float32)
    Wk = np.asarray(Wk, dtype=np.float32)
    Wv = np.asarray(Wv, dtype=np.float32)
    Wo = np.asarray(Wo, dtype=np.float32)

    in_maps = []
    for c in range(NCORES):
        cols = slice(c * CPC, (c + 1) * CPC)
        wq_c = Wq[:, cols].reshape(D, HPC, HD)[:, :, _EVEN_ODD].reshape(D, CPC)
        wk_c = Wk[:, cols].reshape(D, HPC, HD)[:, :, _EVEN_ODD].reshape(D, CPC)
        in_maps.append({
            "xT": xT,
            "wq": np.ascontiguousarray(wq_c.astype(bf)),
            "wk": np.ascontiguousarray(wk_c.astype(bf)),
            "wv": np.ascontiguousarray(Wv[:, cols].astype(bf)),
            "wo": np.ascontiguousarray(Wo[cols, :].astype(bf)),
            "cs1": cs1,
            "cs2": cs2,
        })
    return in_maps


def kernel(x, rope_cos, rope_sin, Wq, Wk, Wv, Wo, _trace=False):
    global last_results
    if "nc" not in _compiled:
        _compiled["nc"] = _build()
    nc = _compiled["nc"]
    in_maps = _prep_inputs(x, rope_cos, rope_sin, Wq, Wk, Wv, Wo)
    res = run_bass_kernel_spmd(
        nc, in_maps, core_ids=list(range(NCORES)), trace=_trace)
    last_results = res
    y = np.sum(np.stack([res.results[c]["y"].astype(np.float32)
                         for c in range(NCORES)]),
               axis=0, dtype=np.float64)
    return y.reshape(B, T, D).astype(np.float32)


# revision 10
# speedup vs baseline: 1.3042x; 1.0180x over previous
"""Tensor-parallel full-attention Bass kernel for TRN2 (v2, mostly-bf16).

Sharding: 16 heads over 8 cores (2 heads/core). Each core computes its heads'
QKV projections, rope, full attention, and its partial output projection
(rows of Wo for its heads); the host sums the 8 partial outputs.

v2 changes vs v1 (549us):
  - bf16 for x, all weights, v, e=exp(s), oT and y partials (half DMA/SBUF,
    FWL weight loads). qT/kT stay f32r for score precision.
  - softmax denominator: DVE/GPSIMD adder tree over e tiles + ONE ones-matmul
    per (h, i-block) instead of a ones-matmul per j-tile (-51us PE).
  - exp over [128,1024] psum pairs (fewer ACT instructions).
  - output-projection (C) matmul groups are software-pipelined one i-block
    late and interleaved between score/PV groups so PE never waits on ACT.
  - y stores + rope swaps on gpsimd DMA queue; x/weights on sync queue so
    the next batch's x prefetches during attention.
  - phase A per block ordered K, V, Q so kT is complete before the last
    Q-rope, letting B start with no PE gap.

Per-core layouts (tokens on the free axis):
  xT   [D=2048, B*T=4096] bf16  x transposed (host-prepped), replicated
  wq/wk [2048, 256] bf16        head-column shard; within each head the 128
                                columns are permuted evens-then-odds so rope
                                pairs become contiguous partition halves
  wv   [2048, 256] bf16         natural column shard
  wo   [256, 2048] bf16         natural row shard
  cs1  [128, 2048] f32          [cos.T ; sin.T] stacked
  cs2  [128, 2048] f32          [sin.T ; cos.T]
"""

import sys

sys.path.insert(0, "/opt/trn_rl_repo")

import numpy as np
import ml_dtypes

import concourse.bass as bass
import concourse.mybir as mybir
import concourse.tile as tile
from concourse import bacc
from concourse.bass_utils import run_bass_kernel_spmd

B, T, D = 2, 2048, 2048
NH, HD = 16, 128
NCORES = 8
HPC = NH // NCORES          # heads per core = 2
CPC = HPC * HD              # proj columns per core = 256
BT = B * T                  # 4096 tokens
P = 128
TBLK = 512                  # phase-A token block
NBLK = T // TBLK            # 4 blocks per batch
DC = D // P                 # 16 contraction chunks
IBLK = 512                  # phase-B query block
NIB = T // IBLK             # 4 i-blocks per batch
NJT = T // P                # 16 key tiles per batch
NJP = NJT // 2              # 8 key-tile pairs
SCALE = 1.0 / float(np.sqrt(HD))

f32 = mybir.dt.float32
f32r = mybir.dt.float32r
bf16 = mybir.dt.bfloat16

_compiled = {}

# exposed for test.py
last_results = None


def _build():
    nc = bacc.Bacc("TRN2", target_bir_lowering=False, debug=False)

    xT_d = nc.dram_tensor("xT", [D, BT], bf16, kind="ExternalInput").ap()
    wq_d = nc.dram_tensor("wq", [D, CPC], bf16, kind="ExternalInput").ap()
    wk_d = nc.dram_tensor("wk", [D, CPC], bf16, kind="ExternalInput").ap()
    wv_d = nc.dram_tensor("wv", [D, CPC], bf16, kind="ExternalInput").ap()
    wo_d = nc.dram_tensor("wo", [CPC, D], bf16, kind="ExternalInput").ap()
    cs1_d = nc.dram_tensor("cs1", [P, T], f32, kind="ExternalInput").ap()
    cs2_d = nc.dram_tensor("cs2", [P, T], f32, kind="ExternalInput").ap()
    y_d = nc.dram_tensor("y", [BT, D], bf16, kind="ExternalOutput").ap()

    with tile.TileContext(nc) as tc:
        _emit(nc, tc, xT_d, wq_d, wk_d, wv_d, wo_d, cs1_d, cs2_d, y_d)
    nc.compile()
    return nc


def _emit(nc, tc, xT_d, wq_d, wk_d, wv_d, wo_d, cs1_d, cs2_d, y_d):
    from contextlib import ExitStack

    Exp = mybir.ActivationFunctionType.Exp
    mult = mybir.AluOpType.mult
    add = mybir.AluOpType.add
    sub = mybir.AluOpType.subtract

    with ExitStack() as ctx:
        const = ctx.enter_context(tc.tile_pool(name="const", bufs=1))
        state = ctx.enter_context(tc.tile_pool(name="state", bufs=1))

        wq_sb = const.tile([P, DC * CPC], bf16, tag="wq")
        wk_sb = const.tile([P, DC * CPC], bf16, tag="wk")
        wv_sb = const.tile([P, DC * CPC], bf16, tag="wv")
        wo_sb = const.tile([P, HPC * D], bf16, tag="wo")
        cs1_sb = const.tile([P, T], f32, tag="cs1")
        cs2_sb = const.tile([P, T], f32, tag="cs2")
        ones_sb = const.tile([P, P], bf16, tag="ones")

        # wk on the sync queue ahead of x block 0; the rest of the consts load
        # in parallel on the scalar engine's DMA queue.
        nc.sync.dma_start(
            wk_sb[:].rearrange("p (dc c) -> p dc c", dc=DC),
            wk_d.rearrange("(dc p) c -> p dc c", p=P))
        nc.scalar.dma_start(cs1_sb[:], cs1_d[:])
        nc.scalar.dma_start(cs2_sb[:], cs2_d[:])
        nc.scalar.dma_start(
            wv_sb[:].rearrange("p (dc c) -> p dc c", dc=DC),
            wv_d.rearrange("(dc p) c -> p dc c", p=P))
        nc.scalar.dma_start(
            wq_sb[:].rearrange("p (dc c) -> p dc c", dc=DC),
            wq_d.rearrange("(dc p) c -> p dc c", p=P))
        nc.scalar.dma_start(
            wo_sb[:].rearrange("p (h d) -> p h d", h=HPC),
            wo_d.rearrange("(h p) d -> p h d", p=P))
        nc.gpsimd.memset(ones_sb[:], 1.0)

        qT_sb = state.tile([P, HPC * T], bf16, tag="qT")
        kT_sb = state.tile([P, HPC * T], bf16, tag="kT")
        v_sb = state.tile([P, NJT * CPC], bf16, tag="v")

        xpool = ctx.enter_context(tc.tile_pool(name="xa", bufs=2))
        yps = ctx.enter_context(tc.tile_pool(name="y_ps", bufs=1, space="PSUM"))
        ypool = ctx.enter_context(tc.tile_pool(name="yb", bufs=3))
        opool = ctx.enter_context(tc.tile_pool(name="ob", bufs=4))
        epool = ctx.enter_context(tc.tile_pool(name="eg", bufs=3))
        rcpool = ctx.enter_context(tc.tile_pool(name="rcg", bufs=2))

        def make_cgroup(g0, tt, oT0, oT1, tl, db):
            def emit():
                yp = yps.tile([P, IBLK], f32, tag="y")
                nc.tensor.matmul(
                    yp[:], oT0[:, tl * P:(tl + 1) * P],
                    wo_sb[:, db * IBLK:(db + 1) * IBLK],
                    start=True, stop=False)
                nc.tensor.matmul(
                    yp[:], oT1[:, tl * P:(tl + 1) * P],
                    wo_sb[:, D + db * IBLK:D + (db + 1) * IBLK],
                    start=False, stop=True)
                yt = ypool.tile([P, IBLK], bf16, tag="yt")
                nc.vector.tensor_copy(out=yt[:], in_=yp[:])
                nc.gpsimd.dma_start(
                    y_d[g0 + tt * P:g0 + (tt + 1) * P,
                        db * IBLK:(db + 1) * IBLK],
                    yt[:])
            return emit

        pending_c = []

        def rope(rpool, pps, t0, dst, h):
            m1 = rpool.tile([P, TBLK], bf16, tag="m1")
            m3 = rpool.tile([P, TBLK], bf16, tag="m3")
            c1 = cs1_sb[:, t0:t0 + TBLK]
            c2 = cs2_sb[:, t0:t0 + TBLK]
            nc.vector.tensor_tensor(m1[:], pps[:], c1, mult)
            nc.vector.tensor_tensor(m3[:], pps[:], c2, mult)
            sw = rpool.tile([P, TBLK], bf16, tag="sw")
            nc.gpsimd.dma_start(sw[0:64, :], m1[64:128, :])
            nc.gpsimd.dma_start(sw[64:128, :], m3[0:64, :])
            o = dst[:, h * T + t0:h * T + t0 + TBLK]
            nc.vector.tensor_tensor(o[0:64, :], m1[0:64, :], sw[0:64, :], sub)
            nc.vector.tensor_tensor(o[64:128, :], m3[64:128, :], sw[64:128, :], add)

        for b in range(B):
            g0 = b * T

            with tc.tile_pool(name=f"ra{b}", bufs=4) as rpool, \
                 tc.tile_pool(name=f"qk_ps{b}", bufs=4, space="PSUM") as qkps, \
                 tc.tile_pool(name=f"v_ps{b}", bufs=2, space="PSUM") as vps:
                for blk in range(NBLK):
                    t0 = blk * TBLK
                    xt = xpool.tile([P, DC * TBLK], bf16, tag="x")
                    nc.sync.dma_start(
                        xt[:].rearrange("p (dc t) -> p dc t", dc=DC),
                        xT_d[:, g0 + t0:g0 + t0 + TBLK]
                        .rearrange("(dc p) t -> p dc t", p=P))

                    if blk == 0 and pending_c:
                        # leftover C groups of the previous batch fill the
                        # x-prefetch window at this batch's start
                        for cg in pending_c:
                            cg()
                        pending_c = []

                    # K first so kT is complete before the last Q rope
                    for w_sb, dst in ((wk_sb, kT_sb), (None, None), (wq_sb, qT_sb)):
                        if w_sb is None:
                            # V-projection: natural [token, col] tiles
                            for half in range(2):
                                vp = vps.tile([P, 2 * CPC], f32, tag="v")
                                for tl2 in range(2):
                                    tl = half * 2 + tl2
                                    for dc in range(DC):
                                        nc.tensor.matmul(
                                            vp[:, tl2 * CPC:(tl2 + 1) * CPC],
                                            xt[:, dc * TBLK + tl * P:
                                               dc * TBLK + (tl + 1) * P],
                                            wv_sb[:, dc * CPC:(dc + 1) * CPC],
                                            start=(dc == 0), stop=(dc == DC - 1))
                                nc.scalar.copy(
                                    v_sb[:, (4 * blk + 2 * half) * CPC:
                                         (4 * blk + 2 * half + 2) * CPC],
                                    vp[:])
                            continue
                        for h in range(HPC):
                            pps = qkps.tile([P, TBLK], f32, tag="qk")
                            for dc in range(DC):
                                nc.tensor.matmul(
                                    pps[:],
                                    w_sb[:, dc * CPC + h * HD:dc * CPC + (h + 1) * HD],
                                    xt[:, dc * TBLK:(dc + 1) * TBLK],
                                    start=(dc == 0), stop=(dc == DC - 1))
                            rope(rpool, pps, t0, dst, h)

            with tc.tile_pool(name=f"o_ps{b}", bufs=2, space="PSUM") as ops, \
                 tc.tile_pool(name=f"d_ps{b}", bufs=1, space="PSUM") as dps, \
                 tc.tile_pool(name=f"s_ps{b}", bufs=2, space="PSUM") as sps:

                def emit_pv_dn(dl):
                    h, jp, e, op, dn = dl["h"], dl["jp"], dl["e"], dl["op"], dl["dn"]
                    nc.tensor.matmul(
                        op[:],
                        v_sb[:, 2 * jp * CPC + h * HD:2 * jp * CPC + (h + 1) * HD],
                        e[:, 0:IBLK], start=(jp == 0), stop=False)
                    nc.tensor.matmul(
                        op[:],
                        v_sb[:, (2 * jp + 1) * CPC + h * HD:
                             (2 * jp + 1) * CPC + (h + 1) * HD],
                        e[:, IBLK:2 * IBLK], start=False, stop=(jp == NJP - 1))
                    nc.tensor.matmul(
                        dn[:], ones_sb[:], e[:, 0:IBLK],
                        start=(jp == 0), stop=False)
                    nc.tensor.matmul(
                        dn[:], ones_sb[:], e[:, IBLK:2 * IBLK],
                        start=False, stop=(jp == NJP - 1))

                def emit_finish(dl):
                    rcp = rcpool.tile([P, IBLK], f32, tag="rc")
                    nc.vector.reciprocal_approx_fast(out=rcp[:], in_=dl["dn"][:])
                    oT_h = opool.tile([P, IBLK], bf16, tag="oT")
                    nc.vector.tensor_tensor(oT_h[:], dl["op"][:], rcp[:], mult)
                    dl["oTs"].append(oT_h)

                def retire(dl):
                    # delayed pv/dn pair; on segment end also the finish and,
                    # on i-block end, queue its output-projection groups
                    emit_pv_dn(dl)
                    if dl["jp"] == NJP - 1:
                        emit_finish(dl)
                        if dl["h"] == HPC - 1:
                            oTs = dl["oTs"]
                            for tl in range(IBLK // P):
                                tt = dl["ib"] * (IBLK // P) + tl
                                for db in range(D // IBLK):
                                    pending_c.append(make_cgroup(
                                        g0, tt, oTs[0], oTs[1], tl, db))

                delayed = None
                for ib in range(NIB):
                    i0 = ib * IBLK
                    cw = pending_c
                    pending_c = []
                    ci = 0
                    oTs = []
                    for h in range(HPC):
                        q_sl = qT_sb[:, h * T + i0:h * T + i0 + IBLK]
                        op = ops.tile([P, IBLK], f32, tag="o")
                        dn = dps.tile([P, IBLK], f32, tag="d")
                        for jp in range(NJP):
                            sidx = h * NJP + jp
                            j0 = h * T + 2 * jp * P
                            sp = sps.tile([P, 2 * IBLK], f32, tag="s")
                            nc.tensor.matmul(
                                sp[:, 0:IBLK], kT_sb[:, j0:j0 + P],
                                q_sl, start=True, stop=True)
                            nc.tensor.matmul(
                                sp[:, IBLK:2 * IBLK],
                                kT_sb[:, j0 + P:j0 + 2 * P],
                                q_sl, start=True, stop=True)
                            e = epool.tile([P, 2 * IBLK], bf16, tag="e")
                            nc.scalar.activation(e[:], sp[:], Exp, scale=SCALE)
                            if sidx >= 2 and ci < len(cw):
                                cw[ci]()
                                ci += 1
                            if delayed is not None:
                                retire(delayed)
                            delayed = dict(h=h, jp=jp, e=e, op=op, dn=dn,
                                           oTs=oTs, ib=ib)
                    while ci < len(cw):
                        cw[ci]()
                        ci += 1
                # drain the last slot of this batch
                if delayed is not None:
                    retire(delayed)
                    delayed = None

        # tail: last i-block's output projection
        for cg in pending_c:
            cg()


_EVEN_ODD = np.concatenate([np.arange(0, HD, 2), np.arange(1, HD, 2)])


def _prep_inputs(x, rope_cos, rope_sin, Wq, Wk, Wv, Wo):
    bf = ml_dtypes.bfloat16
    x = np.asarray(x, dtype=np.float32)
    xT = np.ascontiguousarray(x.reshape(BT, D).T.astype(bf))
    cosT = np.asarray(rope_cos, dtype=np.float32).T
    sinT = np.asarray(rope_sin, dtype=np.float32).T
    cs1 = np.ascontiguousarray(
        np.concatenate([cosT, sinT], axis=0), dtype=np.float32)
    cs2 = np.ascontiguousarray(
        np.concatenate([sinT, cosT], axis=0), dtype=np.float32)
    Wq = np.asarray(Wq, dtype=np.float32)
    Wk = np.asarray(Wk, dtype=np.float32)
    Wv = np.asarray(Wv, dtype=np.float32)
    Wo = np.asarray(Wo, dtype=np.float32)

    in_maps = []
    for c in range(NCORES):
        cols = slice(c * CPC, (c + 1) * CPC)
        wq_c = Wq[:, cols].reshape(D, HPC, HD)[:, :, _EVEN_ODD].reshape(D, CPC)
        wk_c = Wk[:, cols].reshape(D, HPC, HD)[:, :, _EVEN_ODD].reshape(D, CPC)
        in_maps.append({
            "xT": xT,
            "wq": np.ascontiguousarray(wq_c.astype(bf)),
            "wk": np.ascontiguousarray(wk_c.astype(bf)),
            "wv": np.ascontiguousarray(Wv[:, cols].astype(bf)),
            "wo": np.ascontiguousarray(Wo[cols, :].astype(bf)),
            "cs1": cs1,
            "cs2": cs2,
        })
    return in_maps


def kernel(x, rope_cos, rope_sin, Wq, Wk, Wv, Wo, _trace=False):
    global last_results
    if "nc" not in _compiled:
        _compiled["nc"] = _build()
    nc = _compiled["nc"]
    in_maps = _prep_inputs(x, rope_cos, rope_sin, Wq, Wk, Wv, Wo)
    res = run_bass_kernel_spmd(
        nc, in_maps, core_ids=list(range(NCORES)), trace=_trace)
    last_results = res
    y = np.sum(np.stack([res.results[c]["y"].astype(np.float32)
                         for c in range(NCORES)]),
               axis=0, dtype=np.float64)
    return y.reshape(B, T, D).astype(np.float32)
